# revision 1
# baseline (speedup 1.0000x reference)
"""Trainium2 Bass kernel: 16-head MHA forward (B=2, S=2048, D=1024, HD=64).

Sharding: 8 cores, each core owns 2 heads x both batches (head-parallel).
Per core: QKV projection for its heads, fused transposed-score attention
fully on-chip, output projection against its 128 rows of Wo. Host sums the
8 partial outputs and adds bo.

v2 datapath (all matmul operands bf16, PSUM/psum-consumers fp32):
  - scores^T per (head, q-chunk, kj-pair): PE matmul, contraction HD=64
  - exp on ACT (the long pole: 128 x [128,1024] activations ~= 133us)
  - AV with exp-weights STATIONARY and V-augmented (64 cols + ones) MOVING:
    65 moving rows per kj tile instead of 512 -> halves AV PE cost; the
    ones column yields the softmax denominator in psum col 64 for free
  - per-q-row normalization via DVE reciprocal + tensor_scalar_mul
  - vals assembled q-major [128q, 128f], one PE transpose per q-tile,
    then the Wo slice projection (stationary valsT, moving Wo rows)

Emission is slot-scheduled: one exp unit per slot, filler work (proj
chunks of the other batch, V transposes, AV, out-proj) paced between
slots so the PE never waits long on the scores->exp psum rotation and the
ACT engine never starves. Batch-0 head-0 scores are emitted triangularly
against the projection chunks so exps start ~10us into the kernel.

Self-contained: hardcodes shapes; only needs numpy + the concourse stack.
"""

import numpy as np

B, S, D, H, HD = 2, 2048, 1024, 16, 64
NCORES = 8
HPC = H // NCORES          # heads per core = 2
FPC = HPC * 3 * HD         # Wqkv rows per core = 384
VPC = HPC * HD             # value features per core = 128
KD = D // 128              # d-chunks = 8
ST = S // 128              # s-tiles of 128 = 16
SC = S // 512              # s-chunks of 512 = 4

_BUILT = {}


def _build(reps=1):
    if reps in _BUILT:
        return _BUILT[reps]

    import concourse.tile as tile
    import concourse.mybir as mybir
    from concourse import bacc
    from concourse.masks import make_identity

    F32 = mybir.dt.float32
    BF16 = mybir.dt.bfloat16
    EXP = mybir.ActivationFunctionType.Exp

    nc = bacc.Bacc("TRN2", target_bir_lowering=False, debug=False, num_devices=1)

    xT = nc.dram_tensor("xT", [B, D, S], BF16, kind="ExternalInput").ap()
    wqkvT = nc.dram_tensor("wqkvT", [D, FPC], BF16, kind="ExternalInput").ap()
    bq = nc.dram_tensor("bq", [128, 3], F32, kind="ExternalInput").ap()
    woT = nc.dram_tensor("woT", [VPC, D], BF16, kind="ExternalInput").ap()
    outp = nc.dram_tensor("outp", [B, S, D], BF16,
                          kind="ExternalOutput").ap()

    with tile.TileContext(nc) as tc:
        with (
            tc.tile_pool(name="const", bufs=1) as cpool,
            tc.tile_pool(name="sb", bufs=1) as sb,
            tc.tile_pool(name="ps", bufs=1, space="PSUM") as ps,
        ):
            ident = cpool.tile([128, 128], BF16, name="ident")
            make_identity(nc, ident)
            ones16 = nc.const_aps.tensor(1.0, (128, ST), BF16)

            # PE warm-up during the initial DMA wait (p-state ramp): fp32
            # zeros x zeros, ~2048 PE-cycles, ends around first x landing.
            warm_in = cpool.tile([128, 512], F32, name="warm_in")
            nc.vector.memset(warm_in, 0.0)
            for _w, wn in enumerate((512, 256, 256)):
                warm_ps = ps.tile([128, 512], mybir.dt.float32, tag="proj",
                                  bufs=2, name=f"warm_ps{_w}")
                nc.tensor.matmul(warm_ps[:, 0:wn], warm_in[:, 0:128],
                                 warm_in[:, 0:wn], start=True, stop=True)

            bq_sb = cpool.tile([128, 3], F32, name="bq_sb")
            nc.sync.dma_start(out=bq_sb, in_=bq)
            wq_sb = cpool.tile([128, KD, FPC], BF16, name="wq_sb")
            wq_src = wqkvT.rearrange("(k p) f -> p k f", p=128)
            nc.sync.dma_start(out=wq_sb[:, 0:2, :], in_=wq_src[:, 0:2, :])
            wo_sb = cpool.tile([VPC, D], BF16, name="wo_sb")

            for _rep in range(reps):
                qkv = {}     # (b, g, sc) -> [128, 512] bf16; g=0 q, 1 k, 2 v
                             # partitions: [h0 | h1] x 64 dims, type-major
                vaug = {}    # (b, h) -> [128 kj, ST, HD+1] bf16, col HD = 1
                aT = {}      # (b, h, qb, kp) -> [128 kj(2 tiles), 1024] bf16
                vals = {}    # (b, qt) -> [128 q, 128 f] bf16
                valsT = {}   # (b, qt) -> [128 f, 128 q] bf16
                proj_state = {}

                r = f"r{_rep}"

                def x_dma(b, sc, split=False):
                    x_t = sb.tile([128, KD, 512], BF16, tag="xt", bufs=6,
                                  name=f"xt{r}_{b}_{sc}")
                    xr = xT[b].rearrange("(k p) s -> p k s", p=128)
                    ss = slice(sc * 512, (sc + 1) * 512)
                    if split:
                        nc.sync.dma_start(out=x_t[:, 0:4, :],
                                          in_=xr[:, 0:4, ss])
                        return x_t, xr
                    nc.sync.dma_start(out=x_t, in_=xr[:, :, ss])
                    proj_state[(b, sc)] = x_t

                # one projection pass: feature group g (0=q, 1=k, 2=v) for
                # one 512-token chunk; 8 accumulating matmuls + bias-add
                def proj_pass(b, g, sc, k0, k1):
                    x_t = proj_state[(b, sc)]
                    key = (b, g, sc)
                    if k0 == 0:
                        proj_state[key] = ps.tile(
                            [128, 512], mybir.dt.float32, tag="proj", bufs=2,
                            name=f"pp{r}_{b}_{g}_{sc}")
                    pps = proj_state[key]
                    for k in range(k0, k1):
                        nc.tensor.matmul(
                            pps, wq_sb[:, k, g * 128:(g + 1) * 128],
                            x_t[:, k, :], start=(k == 0), stop=(k == KD - 1))
                    if k1 == KD:
                        qkv[key] = sb.tile(
                            [128, 512], BF16, tag=f"qkv{g}", bufs=2 * SC,
                            name=f"qkv{r}_{b}_{g}_{sc}")
                        nc.vector.tensor_scalar_add(
                            qkv[key], pps, bq_sb[:, g:g + 1])

                def vt_init(b, h):
                    va = sb.tile([128, ST, HD + 1], BF16, tag="vaug", bufs=4,
                                 name=f"vaug{r}_{b}_{h}")
                    vaug[(b, h)] = va
                    nc.vector.tensor_copy(va[:, :, HD], ones16)

                def vtrans(b, h, st0, st1):
                    va = vaug[(b, h)]
                    for st in range(st0, st1):
                        pt = ps.tile([128, HD], BF16, tag="sm",
                                     bufs=2, name=f"pt{r}_{b}_{h}_{st}")
                        vsrc = qkv[(b, 2, st // 4)][
                            h * HD:(h + 1) * HD,
                            (st % 4) * 128:(st % 4 + 1) * 128]
                        nc.tensor.transpose(
                            pt, vsrc,
                            ident[h * HD:(h + 1) * HD, h * HD:(h + 1) * HD])
                        nc.vector.tensor_copy(va[:, st, 0:HD], pt)

                def scores_exp(b, h, qb, kp):
                    s_ps = ps.tile([128, 1024], mybir.dt.float32, tag="mm",
                                   bufs=2, name=f"sps{r}_{b}_{h}_{qb}_{kp}")
                    qs = qkv[(b, 0, qb)][h * HD:(h + 1) * HD, :]
                    for i, kj in enumerate((2 * kp, 2 * kp + 1)):
                        kT = qkv[(b, 1, kj // 4)][
                            h * HD:(h + 1) * HD,
                            (kj % 4) * 128:(kj % 4 + 1) * 128]
                        nc.tensor.matmul(s_ps[:, i * 512:(i + 1) * 512],
                                         kT, qs, start=True, stop=True)
                    a = sb.tile([128, 1024], BF16, tag="aT", bufs=36,
                                name=f"aT{r}_{b}_{h}_{qb}_{kp}")
                    aT[(b, h, qb, kp)] = a
                    nc.scalar.activation(a, s_ps, EXP, scale=0.125)

                def av_qt(b, h, qt):
                    qb, ql = qt // 4, qt % 4
                    va = vaug[(b, h)]
                    v_out = ps.tile([128, HD + 1], mybir.dt.float32,
                                    tag="sm", bufs=2,
                                    name=f"vo{r}_{b}_{h}_{qt}")
                    for kj in range(ST):
                        a = aT[(b, h, qb, kj // 2)]
                        col = (kj % 2) * 512 + ql * 128
                        nc.tensor.matmul(v_out, a[:, col:col + 128],
                                         va[:, kj, :],
                                         start=(kj == 0), stop=(kj == ST - 1))
                    inv = sb.tile([128, 1], F32, tag="inv", bufs=4,
                                  name=f"inv{r}_{b}_{h}_{qt}")
                    nc.vector.reciprocal(inv, v_out[:, HD:HD + 1])
                    if (b, qt) not in vals:
                        vals[(b, qt)] = sb.tile(
                            [128, 128], BF16, tag="vals", bufs=2 * ST,
                            name=f"vals{r}_{b}_{qt}")
                    nc.vector.tensor_scalar_mul(
                        vals[(b, qt)][:, h * HD:(h + 1) * HD],
                        v_out[:, 0:HD], inv)

                def vals_trans(b, qt, via_pe=False):
                    valsT[(b, qt)] = sb.tile([128, 128], BF16, tag="valsT",
                                             bufs=2 * ST,
                                             name=f"valsT{r}_{b}_{qt}")
                    if via_pe:
                        # lower-latency path for the kernel tail
                        tp = ps.tile([128, 128], BF16, tag="sm",
                                     bufs=2, name=f"tp{r}_{b}_{qt}")
                        nc.tensor.transpose(tp, vals[(b, qt)], ident)
                        nc.vector.tensor_copy(valsT[(b, qt)], tp)
                    else:
                        nc.sync.dma_start_transpose(
                            out=valsT[(b, qt)], in_=vals[(b, qt)])

                exps_done = [False]
                cms = ("act", "dve")
                ncm = [0]

                def outproj(b, st, jc, cm="dve"):
                    o_ps = ps.tile([128, 512], mybir.dt.float32, tag="proj",
                                   bufs=2, name=f"ops{r}_{b}_{st}_{jc}")
                    nc.tensor.matmul(
                        o_ps, valsT[(b, st)],
                        wo_sb[:, jc * 512:(jc + 1) * 512],
                        start=True, stop=True)
                    o_sb = sb.tile([128, 512], BF16, tag="osb", bufs=4,
                                   name=f"osb{r}_{b}_{st}_{jc}")
                    if cm == "act":
                        nc.scalar.activation(
                            o_sb, o_ps, mybir.ActivationFunctionType.Copy)
                    else:
                        nc.vector.tensor_copy(o_sb, o_ps)
                    nc.sync.dma_start(
                        out=outp[b, st * 128:(st + 1) * 128,
                                 jc * 512:(jc + 1) * 512], in_=o_sb)

                def wo_dma():
                    nc.sync.dma_start(out=wo_sb, in_=woT)

                # ---------------- emission schedule ----------------
                # exp slots: flat qb-major per (b, h) unit; batch-0 head-0
                # overlaps the b0 projection passes (k-chunks first so qb0's
                # scores start early, q/v passes threaded between exp slots).
                flat = [(qb, kp) for qb in range(4) for kp in range(8)]
                # batch 1 alternates heads per q-block so the vals-transpose
                # + out-proj stream spreads over the whole b1 region instead
                # of clustering before the kernel tail
                alt = [(h, qb, kp) for qb in range(4) for h in range(2)
                       for kp in range(8)]
                exp_seq = ([(0, 0) + c for c in flat]
                           + [(0, 1) + c for c in flat]
                           + [(1,) + c for c in alt])
                assert len(exp_seq) == 128

                # filler queue: (id, deps, min_slot, cost, closure)
                Q = []

                def fq(fid, deps, min_slot, cost, fn):
                    Q.append([fid, deps, min_slot, cost, fn])

                # last-exp slot per (b, h, qb)
                done_slot = {}
                for i, (b, h, qb, kp) in enumerate(exp_seq):
                    done_slot[(b, h, qb)] = i

                fq("wo", (), 4, 0, wo_dma)
                # batch-1 projection passes, spread through b0-h1 slots
                fq("xd10", (), 27, 100, lambda: x_dma(1, 0))
                fq("xd11", (), 28, 100, lambda: x_dma(1, 1))
                fq("xd12", (), 32, 100, lambda: x_dma(1, 2))
                fq("xd13", (), 32, 100, lambda: x_dma(1, 3))
                pseq = ([(1, sc) for sc in range(SC)]
                        + [(0, sc) for sc in range(SC)]
                        + [(2, sc) for sc in range(SC)])
                for i, (g, sc) in enumerate(pseq):
                    ms = 28 + 2 * i
                    fq(f"pp1{g}{sc}", (f"xd1{sc}",), ms, 4200,
                       lambda g=g, sc=sc: proj_pass(1, g, sc, 0, KD))
                # V transposes: per 4-tile group, gated on the v chunk
                # (b0 v chunks are emitted at head_pass slots 10/12/18/26)
                b0_vms = (11, 13, 19, 27)
                for b, h in ((0, 0), (0, 1), (1, 0), (1, 1)):
                    ms0 = {(0, 0): 11, (0, 1): 12, (1, 0): 58, (1, 1): 62}[(b, h)]
                    fq(f"vi{b}{h}", (), ms0, 50, lambda b=b, h=h: vt_init(b, h))
                    for g in range(4):
                        deps = (f"vi{b}{h}",)
                        ms = ms0 + g
                        if b == 1:
                            deps += (f"pp12{g}",)
                        else:
                            ms = max(ms, b0_vms[g] + (0 if h == 0 else 1))
                        fq(f"vt{b}{h}{g}", deps, ms, 300,
                           lambda b=b, h=h, g=g: vtrans(b, h, 4 * g, 4 * g + 4))
                # AV + normalization; vals transpose after both heads;
                # outproj. The final q-tiles defer to the tail drain so the
                # last exps run without filler chains; evictions emitted
                # after the final exp alternate ACT (idle by then) and DVE.
                for b in range(2):
                    for qt in range(ST):
                        qb = qt // 4
                        lag = 2 if b == 0 else 3
                        s0 = done_slot[(b, 0, qb)] + lag
                        s1 = done_slot[(b, 1, qb)] + lag
                        tail = (b == 1 and qt >= 12)
                        fq(f"av{b}0{qt}", (f"vt{b}03",),
                           max(s0, 33) if b == 0 else s0, 1150,
                           lambda b=b, qt=qt: av_qt(b, 0, qt))
                        fq(f"av{b}1{qt}", (f"vt{b}13",),
                           129 if tail else s1, 1150,
                           lambda b=b, qt=qt: av_qt(b, 1, qt))
                        fq(f"tr{b}{qt}", (f"av{b}0{qt}", f"av{b}1{qt}"),
                           130 if tail else s1 + 2, 250,
                           lambda b=b, qt=qt: vals_trans(b, qt, via_pe=True))
                        for jc in range(2):
                            if tail:
                                cm = cms[ncm[0] % 2]
                                ncm[0] += 1
                            else:
                                cm = "dve"
                            fq(f"op{b}{qt}{jc}", (f"tr{b}{qt}", "wo"),
                               131 if tail else s1 + 3, 700,
                               lambda b=b, qt=qt, jc=jc, cm=cm:
                                   outproj(b, qt, jc, cm))

                Q.sort(key=lambda it: it[2])
                emitted = set()
                credit = [0.0]

                def pump(slot, budget):
                    credit[0] = min(credit[0] + budget, 4.5 * budget)
                    while credit[0] > 0:
                        pick = None
                        for item in Q:
                            fid, deps, ms, cost, fn = item
                            if ms <= slot and all(d in emitted for d in deps):
                                pick = item
                                break
                        if pick is None:
                            return
                        Q.remove(pick)
                        emitted.add(pick[0])
                        pick[4]()
                        credit[0] -= pick[3]

                def force_emit(fid):
                    if fid in emitted:
                        return
                    item = next(it for it in Q if it[0] == fid)
                    for d in item[1]:
                        force_emit(d)
                    Q.remove(item)
                    emitted.add(fid)
                    item[4]()

                # head: staged DMAs (bq, wq half, x00 half already queued),
                # k0/q0 interleaved at half-pass granularity for DMA overlap,
                # then remaining b0 passes threaded between exp pairs
                x00 = sb.tile([128, KD, 512], BF16, tag="xt", bufs=6,
                              name=f"xt{r}_0_0")
                xr0 = xT[0].rearrange("(k p) s -> p k s", p=128)
                for kc in range(4):
                    ks = slice(2 * kc, 2 * kc + 2)
                    nc.sync.dma_start(out=x00[:, ks, :],
                                      in_=xr0[:, ks, 0:512])
                    if _rep == 0 and kc < 3:
                        ks2 = slice(2 * kc + 2, 2 * kc + 4)
                        nc.sync.dma_start(out=wq_sb[:, ks2, :],
                                          in_=wq_src[:, ks2, :])
                proj_state[(0, 0)] = x00
                for sc in range(1, SC):
                    x_dma(0, sc)
                for kc in range(4):
                    proj_pass(0, 1, 0, 2 * kc, 2 * kc + 2)
                    proj_pass(0, 0, 0, 2 * kc, 2 * kc + 2)
                P0 = lambda g, sc: proj_pass(0, g, sc, 0, KD)
                head_pass = {2: (1, 1), 4: (1, 2), 6: (1, 3), 8: (0, 1),
                             10: (2, 0), 12: (2, 1), 16: (0, 2),
                             18: (2, 2), 24: (0, 3), 26: (2, 3)}
                for slot, (b, h, qb, kp) in enumerate(exp_seq):
                    if slot in head_pass:
                        P0(*head_pass[slot])
                    scores_exp(b, h, qb, kp)
                    pump(slot, 1600)
                # tail: drain the queue in dependency order
                exps_done[0] = True
                guard = 0
                while Q:
                    n0 = len(Q)
                    pump(10 ** 9, 10 ** 9)
                    assert len(Q) < n0 or guard < 3, \
                        f"stuck queue: {[i[0] for i in Q]}"
                    guard += 1

    nc.compile()
    _BUILT[reps] = nc
    return nc


def _in_maps(x, Wqkv, bqkv, Wo):
    import ml_dtypes
    BF = ml_dtypes.bfloat16
    xT = np.ascontiguousarray(x.transpose(0, 2, 1)).astype(BF)
    in_maps = []
    for c in range(NCORES):
        rows = slice(c * FPC, (c + 1) * FPC)
        cols = slice(c * VPC, (c + 1) * VPC)
        # permute head-major [h0:qkv | h1:qkv] rows to type-major
        # [q_h0 q_h1 | k_h0 k_h1 | v_h0 v_h1] so q/k/v of one head share a
        # base partition on chip
        wc = Wqkv[rows].reshape(HPC, 3, HD, D).transpose(1, 0, 2, 3)
        bc = bqkv[rows].reshape(HPC, 3, HD).transpose(1, 0, 2)
        in_maps.append({
            "xT": xT,
            "wqkvT": np.ascontiguousarray(
                wc.reshape(FPC, D).T).astype(BF),
            "bq": np.ascontiguousarray(
                bc.reshape(3, 128).T, dtype=np.float32),
            "woT": np.ascontiguousarray(Wo[:, cols].T).astype(BF),
        })
    return in_maps


def _run_device(x, Wqkv, bqkv, Wo, trace=False):
    from concourse import bass_utils

    nc = _build()
    in_maps = _in_maps(x, Wqkv, bqkv, Wo)
    kw = {}
    if trace:
        kw = dict(trace=True, trace_cores=list(range(NCORES)),
                  stitch_traces=True)
    res = bass_utils.run_bass_kernel_spmd(
        nc, in_maps, core_ids=list(range(NCORES)), **kw)
    acc = res.results[0]["outp"].astype(np.float64)
    for c in range(1, NCORES):
        acc += res.results[c]["outp"]
    return acc, res


def _numpy_fallback(x, mask, Wqkv, bqkv, Wo, bo):
    qkv = x @ Wqkv.T + bqkv
    qkv = qkv.reshape(B, S, H, 3 * HD).transpose(0, 2, 1, 3)
    q, k, v = np.split(qkv, 3, axis=-1)
    sc = np.einsum("bhqd,bhkd->bhqk", q, k) / np.sqrt(HD).astype(np.float32)
    sc = sc + mask
    sc = sc - sc.max(axis=-1, keepdims=True)
    a = np.exp(sc)
    a /= a.sum(axis=-1, keepdims=True)
    vals = np.einsum("bhqk,bhkd->bhqd", a, v)
    vals = vals.transpose(0, 2, 1, 3).reshape(B, S, D)
    return (vals @ Wo.T + bo).astype(np.float32)


def kernel(x, mask, Wqkv, bqkv, Wo, bo):
    x = np.asarray(x, dtype=np.float32)
    mask = np.asarray(mask, dtype=np.float32)
    Wqkv = np.asarray(Wqkv, dtype=np.float32)
    bqkv = np.asarray(bqkv, dtype=np.float32)
    Wo = np.asarray(Wo, dtype=np.float32)
    bo = np.asarray(bo, dtype=np.float32)
    if mask.any():
        # device kernel folds the (all-zero) mask away; fall back if nonzero
        return _numpy_fallback(x, mask, Wqkv, bqkv, Wo, bo)
    acc, _ = _run_device(x, Wqkv, bqkv, Wo)
    return (acc + bo).astype(np.float32)

